# revision 5
# baseline (speedup 1.0000x reference)
"""DFHGNN Trainium2 kernel: 8-way node-sharded hypergraph conv network.

Sharding: rows (nodes) of x/z/incidence split across 8 cores (2500 rows each,
zero-padded to 2560). Weight matrices replicated. The H^T X edge aggregation
is computed per-shard and AllReduce-summed; the H @ He scatter is local.
Incidence operands run in fp16 (accuracy checked: ~5e-5 max rel err), all
accumulation in fp32.
"""

import sys

sys.path.insert(0, "/opt/trn_rl_repo")

import numpy as np

import concourse.bass as bass
import concourse.mybir as mybir
import concourse.tile as tile
from concourse import bacc
from concourse.masks import make_identity

P = 128
CORES = 8
NSH = 2500          # nodes per shard
NP = 2560           # padded nodes per shard
NT = NP // P        # 20 node tiles
NCH = NP // 512     # 5 node chunks of 512
E = 4096
ET = E // P         # 32 edge tiles
EG = 8              # edge groups of 512
IN_DIM, DET_DIM, HID, HALF, OUT = 512, 256, 256, 128, 2
EPS = 1e-6

F32 = mybir.dt.float32
F16 = mybir.dt.float16
AF = mybir.ActivationFunctionType

_CACHE = {}
RG_SPLIT = False


def RG():
    if RG_SPLIT:
        return [[i] for i in range(CORES)]
    return [list(range(CORES))]


def build_program():
    nc = bacc.Bacc("TRN2", target_bir_lowering=False)

    # ---- DRAM I/O ----
    xT = nc.dram_tensor("xT", [IN_DIM, NP], F32, kind="ExternalInput")
    zT = nc.dram_tensor("zT", [DET_DIM, NP], F32, kind="ExternalInput")
    h16 = nc.dram_tensor("h16", [NP, E], F16, kind="ExternalInput")
    ht16 = nc.dram_tensor("ht16", [E, NP], F16, kind="ExternalInput")
    w16 = nc.dram_tensor("w16", [P, ET], F16, kind="ExternalInput")
    wf = nc.dram_tensor("wf", [P, ET], F32, kind="ExternalInput")
    psi_w = nc.dram_tensor("psi_w", [IN_DIM, HALF], F32, kind="ExternalInput")
    phi_w = nc.dram_tensor("phi_w", [DET_DIM, HALF], F32, kind="ExternalInput")
    g1_w = nc.dram_tensor("g1_w", [2 * HALF, HID], F32, kind="ExternalInput")
    g2_w = nc.dram_tensor("g2_w", [HID, HALF], F32, kind="ExternalInput")
    th1 = nc.dram_tensor("th1", [HALF, HID], F32, kind="ExternalInput")
    th2 = nc.dram_tensor("th2", [HID, HID], F32, kind="ExternalInput")
    out_w = nc.dram_tensor("out_w", [HID, OUT], F32, kind="ExternalInput")
    psi_b = nc.dram_tensor("psi_b", [P, 1], F32, kind="ExternalInput")
    phi_b = nc.dram_tensor("phi_b", [P, 1], F32, kind="ExternalInput")
    g1_b = nc.dram_tensor("g1_b", [P, 2], F32, kind="ExternalInput")
    g2_b = nc.dram_tensor("g2_b", [P, 1], F32, kind="ExternalInput")
    b1 = nc.dram_tensor("b1", [P, 2], F32, kind="ExternalInput")
    b2 = nc.dram_tensor("b2", [P, 2], F32, kind="ExternalInput")
    out_b = nc.dram_tensor("out_b", [OUT, 1], F32, kind="ExternalInput")

    logits_t = nc.dram_tensor("logits_t", [OUT, NP], F32, kind="ExternalOutput")
    gate_t = nc.dram_tensor("gate_t", [HALF, NP], F32, kind="ExternalOutput")

    with tile.TileContext(nc) as tc:
        with (
            tc.tile_pool(name="wpool", bufs=1) as wpool,
            tc.tile_pool(name="big", bufs=1) as big,
            tc.tile_pool(name="xz", bufs=3) as xzp,
            tc.tile_pool(name="hp", bufs=4) as hp,
            tc.tile_pool(name="htp", bufs=6) as htp,
            tc.tile_pool(name="xsp", bufs=20) as xsp,
            tc.tile_pool(name="hescp", bufs=32) as hescp,
            tc.tile_pool(name="drp", bufs=4) as drp,
            tc.tile_pool(name="tmp", bufs=2) as tmp,
            tc.tile_pool(name="ps_acc", bufs=4, space="PSUM") as ps_acc,
            tc.tile_pool(name="ps_mm", bufs=2, space="PSUM") as ps_mm,
            tc.tile_pool(name="ps_tp", bufs=2, space="PSUM") as ps_tp,
            tc.tile_pool(name="dram", bufs=1, space="DRAM") as dram,
        ):
            # ---- weight / bias / const loads ----
            psi_sb = wpool.tile([P, 4, HALF], F32, tag="psi")
            nc.sync.dma_start(psi_sb[:], psi_w.rearrange("(ko p) m -> p ko m", p=P))
            phi_sb = wpool.tile([P, 2, HALF], F32, tag="phi")
            nc.sync.dma_start(phi_sb[:], phi_w.rearrange("(ko p) m -> p ko m", p=P))
            g1_sb = wpool.tile([P, 2, HID], F32, tag="g1")
            nc.sync.dma_start(g1_sb[:], g1_w.rearrange("(ko p) m -> p ko m", p=P))
            g2_sb = wpool.tile([P, 2, HALF], F32, tag="g2")
            nc.sync.dma_start(g2_sb[:], g2_w.rearrange("(ko p) m -> p ko m", p=P))
            th1_sb = wpool.tile([P, HID], F32, tag="th1")
            nc.sync.dma_start(th1_sb[:], th1[:])
            th2_sb = wpool.tile([P, 2, HID], F32, tag="th2")
            nc.sync.dma_start(th2_sb[:], th2.rearrange("(ko p) m -> p ko m", p=P))
            outw_sb = wpool.tile([P, 2, OUT], F32, tag="outw")
            nc.sync.dma_start(outw_sb[:], out_w.rearrange("(ko p) m -> p ko m", p=P))
            w16_sb = wpool.tile([P, ET], F16, tag="w16")
            nc.sync.dma_start(w16_sb[:], w16[:])
            wf_sb = wpool.tile([P, ET], F32, tag="wf")
            nc.sync.dma_start(wf_sb[:], wf[:])
            psib = wpool.tile([P, 1], F32, tag="psib")
            nc.sync.dma_start(psib[:], psi_b[:])
            phib = wpool.tile([P, 1], F32, tag="phib")
            nc.sync.dma_start(phib[:], phi_b[:])
            g1b = wpool.tile([P, 2], F32, tag="g1b")
            nc.sync.dma_start(g1b[:], g1_b[:])
            g2b = wpool.tile([P, 1], F32, tag="g2b")
            nc.sync.dma_start(g2b[:], g2_b[:])
            b1sb = wpool.tile([P, 2], F32, tag="b1sb")
            nc.sync.dma_start(b1sb[:], b1[:])
            b2sb = wpool.tile([P, 2], F32, tag="b2sb")
            nc.sync.dma_start(b2sb[:], b2[:])
            outb = wpool.tile([OUT, 1], F32, tag="outb")
            nc.sync.dma_start(outb[:], out_b[:])
            ident = wpool.tile([P, P], F16, tag="ident")
            make_identity(nc, ident[:])
            wde = wpool.tile([P, ET], F32, tag="wde")

            dv_dram = dram.tile([1, NP], F32)
            ar1_in = dram.tile([E, HID + 1], F32)
            ar1_outs = [dram.tile([512, HID + 1], F32, addr_space="Shared",
                                  name=f"ar1o_{g}") for g in range(EG)]
            ar2_in = dram.tile([E, HID], F32)
            ar2_outs = [dram.tile([512, HID], F32, addr_space="Shared",
                                  name=f"ar2o_{g}") for g in range(EG)]

            # ---- Dv pass: Dv^T[1, n] = w^T @ H^T, then isv = rsqrt(max(Dv, eps))
            for c in range(NCH):
                sl = bass.ts(c, 512)
                dv_ps = ps_tp.tile([1, 512], F32, tag="tp")
                for et in range(ET):
                    htc = htp.tile([P, 512], F16, tag="ht")
                    nc.sync.dma_start(htc[:], ht16[et * P:(et + 1) * P, sl])
                    nc.tensor.matmul(dv_ps[:], w16_sb[:, et:et + 1], htc[:],
                                     start=(et == 0), stop=(et == ET - 1))
                dv_sb = tmp.tile([1, 512], F32, tag="dvrow")
                nc.vector.tensor_scalar_max(dv_sb[:], dv_ps[:], EPS)
                nc.vector.reciprocal(dv_sb[:], dv_sb[:])
                nc.scalar.activation(dv_sb[:], dv_sb[:], AF.Sqrt)
                nc.sync.dma_start(dv_dram[:, sl], dv_sb[:])

            isvb = big.tile([P, NP], F32, tag="isvb")
            nc.sync.dma_start(isvb[:], dv_dram[:].to_broadcast((P, NP)))

            # ---- stage A (per 512-node chunk, feature-major) + Xs1 prep ----
            xs1 = {}
            for c in range(NCH):
                sl = bass.ts(c, 512)
                # proj_x^T
                ps = ps_mm.tile([P, 512], F32, tag="mm")
                for ko in range(4):
                    xc = xzp.tile([P, 512], F32, tag="xz")
                    nc.sync.dma_start(xc[:], xT[ko * P:(ko + 1) * P, sl])
                    nc.tensor.matmul(ps[:], psi_sb[:, ko], xc[:],
                                     start=(ko == 0), stop=(ko == 3))
                pxc = tmp.tile([P, 512], F32, tag="pxc")
                nc.scalar.activation(pxc[:], ps[:], AF.Identity, bias=psib[:])
                # proj_z^T
                ps2 = ps_mm.tile([P, 512], F32, tag="mm")
                for ko in range(2):
                    zc = xzp.tile([P, 512], F32, tag="xz")
                    nc.sync.dma_start(zc[:], zT[ko * P:(ko + 1) * P, sl])
                    nc.tensor.matmul(ps2[:], phi_sb[:, ko], zc[:],
                                     start=(ko == 0), stop=(ko == 1))
                pzc = tmp.tile([P, 512], F32, tag="pzc")
                nc.scalar.activation(pzc[:], ps2[:], AF.Identity, bias=phib[:])
                # gate hidden r = relu(g1_w^T cat + g1_b)
                rc = []
                for mo in range(2):
                    ps3 = ps_mm.tile([P, 512], F32, tag="mm")
                    nc.tensor.matmul(ps3[:], g1_sb[:, 0, bass.ts(mo, P)], pxc[:],
                                     start=True, stop=False)
                    nc.tensor.matmul(ps3[:], g1_sb[:, 1, bass.ts(mo, P)], pzc[:],
                                     start=False, stop=True)
                    r = tmp.tile([P, 512], F32, tag=f"rc{mo}")
                    nc.scalar.activation(r[:], ps3[:], AF.Relu, bias=g1b[:, mo:mo + 1])
                    rc.append(r)
                # gate = sigmoid(g2_w^T r + g2_b)
                ps4 = ps_mm.tile([P, 512], F32, tag="mm")
                nc.tensor.matmul(ps4[:], g2_sb[:, 0], rc[0][:], start=True, stop=False)
                nc.tensor.matmul(ps4[:], g2_sb[:, 1], rc[1][:], start=False, stop=True)
                gc = tmp.tile([P, 512], F32, tag="gatec")
                nc.scalar.activation(gc[:], ps4[:], AF.Sigmoid, bias=g2b[:])
                nc.sync.dma_start(gate_t[:, sl], gc[:])
                # fused = px + gate * (pz - px)
                fc = tmp.tile([P, 512], F32, tag="fusedc")
                nc.vector.tensor_sub(fc[:], pzc[:], pxc[:])
                nc.vector.tensor_mul(fc[:], fc[:], gc[:])
                nc.vector.tensor_add(fc[:], fc[:], pxc[:])
                # Xt1^T = th1^T fused ; Xs1 = (Xt1 * isv) transposed to node-major
                for mo in range(2):
                    psx = ps_mm.tile([P, 512], F32, tag="mm")
                    nc.tensor.matmul(psx[:], th1_sb[:, bass.ts(mo, P)], fc[:],
                                     start=True, stop=True)
                    xst = tmp.tile([P, 512], F16, tag="xst16")
                    nc.vector.tensor_mul(xst[:], psx[:], isvb[:, sl])
                    for s in range(4):
                        o = c * 4 + s
                        if mo == 0:
                            xs1[o] = xsp.tile([P, HID + 1], F16, tag="xs", bufs=20, name=f"xs1_{o}")
                            nc.vector.memset(xs1[o][:, HID:HID + 1], 1.0)
                        tp_ps = ps_tp.tile([P, P], F16, tag="tp")
                        nc.tensor.transpose(tp_ps[:], xst[:, bass.ts(s, P)], ident[:])
                        nc.vector.tensor_copy(xs1[o][:, bass.ts(mo, P)], tp_ps[:])

            # ---- gather1: He1[e, f] (+De in col HID) = sum_n H[n,e] * Xs1[n, f|1]
            for g in range(EG):
                he_ps = [ps_acc.tile([P, HID + 1], F32, tag="acc", name=f"he_ps_{g}_{t}") for t in range(4)]
                for n in range(NT):
                    hn = hp.tile([P, 512], F16, tag="h")
                    nc.sync.dma_start(hn[:], h16[n * P:(n + 1) * P, bass.ts(g, 512)])
                    for t in range(4):
                        nc.tensor.matmul(he_ps[t][:], hn[:, bass.ts(t, P)], xs1[n][:],
                                         start=(n == 0), stop=(n == NT - 1))
                for t in range(4):
                    d = drp.tile([P, HID + 1], F32, tag="dr")
                    nc.vector.tensor_copy(d[:], he_ps[t][:])
                    row = (g * 4 + t) * P
                    nc.sync.dma_start(ar1_in[row:row + P, :], d[:])
                nc.gpsimd.collective_compute(
                    "AllReduce", mybir.AluOpType.add,
                    replica_groups=RG(),
                    ins=[ar1_in[g * 512:(g + 1) * 512, :].opt()],
                    outs=[ar1_outs[g].opt()],
                )

            # ---- He1 scaling: He_sc = He * (w / max(De, eps)); cache wde
            he1 = {}
            for et in range(ET):
                d = drp.tile([P, HID + 1], F32, tag="dr")
                nc.sync.dma_start(d[:], ar1_outs[et // 4][(et % 4) * P:(et % 4 + 1) * P, :])
                det = tmp.tile([P, 1], F32, tag="det")
                nc.vector.tensor_scalar_max(det[:], d[:, HID:HID + 1], EPS)
                nc.vector.reciprocal(det[:], det[:])
                nc.vector.tensor_mul(wde[:, et:et + 1], wf_sb[:, et:et + 1], det[:])
                he1[et] = hescp.tile([P, HID], F16, tag="hesc", bufs=32, name=f"he1_{et}")
                nc.vector.tensor_mul(he1[et][:], d[:, :HID],
                                     wde[:, et:et + 1].to_broadcast((P, HID)))

            # ---- scatter1 (h1^T = relu((H @ He_sc1)^T * isv + b1)) + Xs2 prep
            xs2 = {}
            for c in range(NCH):
                sl = bass.ts(c, 512)
                spa = ps_mm.tile([P, 512], F32, tag="mm")
                spb = ps_mm.tile([P, 512], F32, tag="mm")
                for et in range(ET):
                    htc = htp.tile([P, 512], F16, tag="ht")
                    nc.sync.dma_start(htc[:], ht16[et * P:(et + 1) * P, sl])
                    nc.tensor.matmul(spa[:], he1[et][:, :P], htc[:],
                                     start=(et == 0), stop=(et == ET - 1))
                    nc.tensor.matmul(spb[:], he1[et][:, P:], htc[:],
                                     start=(et == 0), stop=(et == ET - 1))
                h1c = []
                for f, sp in enumerate([spa, spb]):
                    t = tmp.tile([P, 512], F32, tag=f"sc{f}")
                    nc.vector.tensor_mul(t[:], sp[:], isvb[:, sl])
                    h = tmp.tile([P, 512], F32, tag=f"h1c{f}")
                    nc.scalar.activation(h[:], t[:], AF.Relu, bias=b1sb[:, f:f + 1])
                    h1c.append(h)
                # Xt2^T = th2^T h1 ; Xs2 node-major
                for mo in range(2):
                    psx = ps_tp.tile([P, 512], F32, tag="tp")
                    nc.tensor.matmul(psx[:], th2_sb[:, 0, bass.ts(mo, P)], h1c[0][:],
                                     start=True, stop=False)
                    nc.tensor.matmul(psx[:], th2_sb[:, 1, bass.ts(mo, P)], h1c[1][:],
                                     start=False, stop=True)
                    xst = tmp.tile([P, 512], F16, tag="xst16")
                    nc.vector.tensor_mul(xst[:], psx[:], isvb[:, sl])
                    for s in range(4):
                        o = c * 4 + s
                        if mo == 0:
                            xs2[o] = xsp.tile([P, HID + 1], F16, tag="xs", name=f"xs2_{o}")
                        tp_ps = ps_tp.tile([P, P], F16, tag="tp")
                        nc.tensor.transpose(tp_ps[:], xst[:, bass.ts(s, P)], ident[:])
                        nc.vector.tensor_copy(xs2[o][:, bass.ts(mo, P)], tp_ps[:])

            # ---- gather2
            for g in range(EG):
                he_ps = [ps_acc.tile([P, HID], F32, tag="acc", name=f"he2_ps_{g}_{t}") for t in range(4)]
                for n in range(NT):
                    hn = hp.tile([P, 512], F16, tag="h")
                    nc.sync.dma_start(hn[:], h16[n * P:(n + 1) * P, bass.ts(g, 512)])
                    for t in range(4):
                        nc.tensor.matmul(he_ps[t][:], hn[:, bass.ts(t, P)],
                                         xs2[n][:, :HID],
                                         start=(n == 0), stop=(n == NT - 1))
                for t in range(4):
                    d = drp.tile([P, HID], F32, tag="dr")
                    nc.vector.tensor_copy(d[:], he_ps[t][:])
                    row = (g * 4 + t) * P
                    nc.sync.dma_start(ar2_in[row:row + P, :], d[:])
                nc.gpsimd.collective_compute(
                    "AllReduce", mybir.AluOpType.add,
                    replica_groups=RG(),
                    ins=[ar2_in[g * 512:(g + 1) * 512, :].opt()],
                    outs=[ar2_outs[g].opt()],
                )

            he2 = {}
            for et in range(ET):
                d = drp.tile([P, HID], F32, tag="dr")
                nc.sync.dma_start(d[:], ar2_outs[et // 4][(et % 4) * P:(et % 4 + 1) * P, :])
                he2[et] = hescp.tile([P, HID], F16, tag="hesc", name=f"he2_{et}")
                nc.vector.tensor_mul(he2[et][:], d[:],
                                     wde[:, et:et + 1].to_broadcast((P, HID)))

            # ---- scatter2 + logits
            for c in range(NCH):
                sl = bass.ts(c, 512)
                spa = ps_mm.tile([P, 512], F32, tag="mm")
                spb = ps_mm.tile([P, 512], F32, tag="mm")
                for et in range(ET):
                    htc = htp.tile([P, 512], F16, tag="ht")
                    nc.sync.dma_start(htc[:], ht16[et * P:(et + 1) * P, sl])
                    nc.tensor.matmul(spa[:], he2[et][:, :P], htc[:],
                                     start=(et == 0), stop=(et == ET - 1))
                    nc.tensor.matmul(spb[:], he2[et][:, P:], htc[:],
                                     start=(et == 0), stop=(et == ET - 1))
                h2c = []
                for f, sp in enumerate([spa, spb]):
                    t = tmp.tile([P, 512], F32, tag=f"sc{f}")
                    nc.vector.tensor_mul(t[:], sp[:], isvb[:, sl])
                    h = tmp.tile([P, 512], F32, tag=f"h2c{f}")
                    nc.scalar.activation(h[:], t[:], AF.Relu, bias=b2sb[:, f:f + 1])
                    h2c.append(h)
                lg_ps = ps_tp.tile([OUT, 512], F32, tag="tp")
                nc.tensor.matmul(lg_ps[:], outw_sb[:, 0], h2c[0][:],
                                 start=True, stop=False)
                nc.tensor.matmul(lg_ps[:], outw_sb[:, 1], h2c[1][:],
                                 start=False, stop=True)
                lg = tmp.tile([OUT, 512], F32, tag="lg")
                nc.scalar.activation(lg[:], lg_ps[:], AF.Identity, bias=outb[:])
                nc.sync.dma_start(logits_t[:, sl], lg[:])

    nc.compile()
    return nc


def prep_in_maps(inputs):
    """Shard + transpose + cast full inputs into per-core in_maps."""
    x = np.asarray(inputs["x"], np.float32)
    z = np.asarray(inputs["z"], np.float32)
    H = np.asarray(inputs["incidence"], np.float32)
    w = np.asarray(inputs["edge_weights"], np.float32)

    wf = np.ascontiguousarray(w.reshape(ET, P).T)
    w16 = wf.astype(np.float16)
    shared = {
        "w16": w16, "wf": wf,
        "psi_w": np.asarray(inputs["psi_w"], np.float32),
        "phi_w": np.asarray(inputs["phi_w"], np.float32),
        "g1_w": np.asarray(inputs["g1_w"], np.float32),
        "g2_w": np.asarray(inputs["g2_w"], np.float32),
        "th1": np.asarray(inputs["th1"], np.float32),
        "th2": np.asarray(inputs["th2"], np.float32),
        "out_w": np.asarray(inputs["out_w"], np.float32),
        "psi_b": np.asarray(inputs["psi_b"], np.float32).reshape(P, 1),
        "phi_b": np.asarray(inputs["phi_b"], np.float32).reshape(P, 1),
        "g1_b": np.ascontiguousarray(
            np.asarray(inputs["g1_b"], np.float32).reshape(2, P).T),
        "g2_b": np.asarray(inputs["g2_b"], np.float32).reshape(P, 1),
        "b1": np.ascontiguousarray(
            np.asarray(inputs["b1"], np.float32).reshape(2, P).T),
        "b2": np.ascontiguousarray(
            np.asarray(inputs["b2"], np.float32).reshape(2, P).T),
        "out_b": np.asarray(inputs["out_b"], np.float32).reshape(OUT, 1),
    }

    in_maps = []
    for i in range(CORES):
        rows = slice(i * NSH, (i + 1) * NSH)
        xT = np.zeros((IN_DIM, NP), np.float32)
        xT[:, :NSH] = x[rows].T
        zT = np.zeros((DET_DIM, NP), np.float32)
        zT[:, :NSH] = z[rows].T
        h16 = np.zeros((NP, E), np.float16)
        h16[:NSH] = H[rows].astype(np.float16)
        ht16 = np.ascontiguousarray(h16.T)
        in_maps.append({"xT": xT, "zT": zT, "h16": h16, "ht16": ht16, **shared})
    return in_maps


def assemble_outputs(results):
    logits = np.concatenate(
        [results[i]["logits_t"].T[:NSH] for i in range(CORES)], axis=0)
    gate = np.concatenate(
        [results[i]["gate_t"].T[:NSH] for i in range(CORES)], axis=0)
    return logits, gate


def _get_nc():
    if "nc" not in _CACHE:
        _CACHE["nc"] = build_program()
    return _CACHE["nc"]


def kernel(**inputs):
    from concourse import bass2jax

    nc = _get_nc()
    in_maps = prep_in_maps(inputs)
    results = bass2jax.run_bass_via_pjrt(nc, in_maps, n_cores=CORES)
    return assemble_outputs(results)
